# revision 1
# baseline (speedup 1.0000x reference)
"""Trainium2 Bass kernel for nn_ConvGraphSelfLoop.

out = where(any(adj>=0, axes -1,-2), relu(features @ W + b), features)

Sharding: B*V = 65536 vertices split evenly across 8 NeuronCores (8192
each); W/bias replicated; no cross-core communication.

Per core (fully unrolled, 64 token-tiles of 128 vertices):
  - PE transposes x [128,1024] -> xT chunks (fp32, exact) into PSUM
  - ACT evicts PSUM -> SBUF rounding to float32r (1 cyc/row matmul rate)
  - PE: per 512-wide u-half: K=1 bias matmul + 8 accumulated f32r matmuls
  - ACT: relu(psum * mask) eviction (mask zeroes invalid rows)
  - DVE: mask from adjacency; xc = x*(1-mask); out = relu_part + xc
x is loaded twice (separate tiles for the PE and DVE consumers) to keep
DMA WAR chains simple. Built on bacc.Bacc: compile() legalizes the
1-wait-per-instruction TRN2 constraint (split_sync_waits).
"""
import numpy as np
import concourse.bass as bass
import concourse.bacc as bacc
import concourse.mybir as mybir
import concourse.tile as tile
from concourse.bass_utils import run_bass_kernel_spmd

B, V, E, NN = 4, 16384, 4, 32
F, U = 1024, 1024
NCORES = 8
T = B * V // NCORES          # 8192 tokens per core
P = 128
NT = T // P                  # 64 token tiles
C = F // P                   # 8 contraction chunks
NH = U // 512                # 2 u-halves
BUFS = 3

f32 = mybir.dt.float32
f32r = mybir.dt.float32r
i32 = mybir.dt.int32
AF = mybir.ActivationFunctionType
ALU = mybir.AluOpType


def _build():
    nc = bacc.Bacc("TRN2", target_bir_lowering=False, debug=False,
                   num_devices=NCORES)
    feat_d = nc.dram_tensor("features", [T, F], f32, kind="ExternalInput")
    adj_d = nc.dram_tensor("adjacency", [T, E * NN], i32, kind="ExternalInput")
    w_d = nc.dram_tensor("weight", [F, U], f32, kind="ExternalInput")
    bias_d = nc.dram_tensor("bias", [1, U], f32, kind="ExternalInput")
    out_d = nc.dram_tensor("out", [T, U], f32, kind="ExternalOutput")

    with tile.TileContext(nc) as tc:
        with tc.tile_pool(name="const", bufs=1) as const, \
             tc.tile_pool(name="xp", bufs=BUFS) as xp, \
             tc.tile_pool(name="xd", bufs=BUFS) as xd, \
             tc.tile_pool(name="xtp", bufs=BUFS) as xtp, \
             tc.tile_pool(name="op", bufs=BUFS) as op, \
             tc.tile_pool(name="ap", bufs=BUFS) as apool, \
             tc.tile_pool(name="mp", bufs=BUFS) as mp, \
             tc.tile_pool(name="psT", bufs=2, space="PSUM") as psT, \
             tc.tile_pool(name="psO", bufs=2, space="PSUM") as psO:

            # ---- startup constants ----
            w_st = const.tile([P, C * U], f32)
            for c in range(C):
                nc.sync.dma_start(w_st[:, c * U:(c + 1) * U],
                                  w_d.ap()[c * P:(c + 1) * P, :])
            w_r = const.tile([P, C * U], f32r)
            for c in range(C):
                nc.scalar.copy(w_r[:, c * U:(c + 1) * U],
                               w_st[:, c * U:(c + 1) * U])
            bias_st = const.tile([1, U], f32)
            nc.sync.dma_start(bias_st[:], bias_d.ap())
            bias_r = const.tile([1, U], f32r)
            nc.scalar.copy(bias_r[:], bias_st[:])
            ones_st = const.tile([1, P], f32)
            nc.gpsimd.memset(ones_st[:], 1.0)
            ones_r = const.tile([1, P], f32r)
            nc.scalar.copy(ones_r[:], ones_st[:])
            ident = const.tile([P, P], f32)
            nc.gpsimd.memset(ident[:], 0.0)
            nc.gpsimd.affine_select(
                out=ident[:], in_=ident[:],
                compare_op=ALU.not_equal, fill=1.0, base=0,
                pattern=[[-1, P]], channel_multiplier=1,
            )

            for t in range(NT):
                rows = slice(t * P, (t + 1) * P)
                # ---- DMA loads ----
                x_pe = xp.tile([P, F], f32, tag="x_pe")
                nc.sync.dma_start(x_pe[:], feat_d.ap()[rows, :])
                x_dve = xd.tile([P, F], f32, tag="x_dve")
                nc.sync.dma_start(x_dve[:], feat_d.ap()[rows, :])
                adj_t = apool.tile([P, E * NN], i32, tag="adj")
                nc.sync.dma_start(adj_t[:], adj_d.ap()[rows, :])

                # ---- PE: bias matmuls open the psum accumulation ----
                po = psO.tile([P, U], f32, tag="po")
                for h in range(NH):
                    nc.tensor.matmul(po[:, h * 512:(h + 1) * 512],
                                     ones_r[:], bias_r[:, h * 512:(h + 1) * 512],
                                     start=True, stop=False)

                # ---- PE: transposes (fp32 exact) ----
                pT = psT.tile([P, 2 * 512], f32, tag="pT")
                for c in range(C):
                    nc.tensor.transpose(pT[:, c * P:(c + 1) * P],
                                        x_pe[:, c * P:(c + 1) * P], ident[:])

                # ---- ACT: evict transposes to SBUF as f32r (one big copy) ----
                xT_r = xtp.tile([P, F], f32r, tag="xT_r")
                nc.scalar.copy(xT_r[:], pT[:])

                # ---- DVE: mask pipeline ----
                mx = mp.tile([P, 1], i32, tag="mx")
                nc.vector.tensor_reduce(mx[:], adj_t[:],
                                        axis=mybir.AxisListType.X, op=ALU.max)
                m_f = mp.tile([P, 1], f32, tag="m_f")
                nc.vector.tensor_scalar(m_f[:], mx[:], 0, None, ALU.is_ge)
                minv = mp.tile([P, 1], f32, tag="minv")
                nc.vector.tensor_scalar(minv[:], m_f[:], -1.0, 1.0,
                                        ALU.mult, ALU.add)
                xc = xd.tile([P, F], f32, tag="xc")
                nc.vector.tensor_scalar(xc[:], x_dve[:], minv[:], None, ALU.mult)

                # ---- ACT: mask copy (washes DVE dep into ACT stream) ----
                m_act = mp.tile([P, 1], f32, tag="m_act")
                nc.scalar.copy(m_act[:], m_f[:])

                # ---- PE: main f32r matmuls ----
                for h in range(NH):
                    for c in range(C):
                        nc.tensor.matmul(
                            po[:, h * 512:(h + 1) * 512],
                            xT_r[:, c * P:(c + 1) * P],
                            w_r[:, c * U + h * 512: c * U + (h + 1) * 512],
                            start=False, stop=(c == C - 1))

                # ---- ACT: relu(psum * mask) -> r_t ----
                r_t = op.tile([P, U], f32, tag="r_t")
                nc.scalar.activation(r_t[:], po[:], AF.Relu, scale=m_act[:])

                # ---- DVE: out = r_t + xc ----
                out_t = op.tile([P, U], f32, tag="out_t")
                nc.vector.tensor_tensor(out=out_t[:], in0=r_t[:], in1=xc[:],
                                        op=ALU.add)

                # ---- DMA store ----
                nc.sync.dma_start(out_d.ap()[rows, :], out_t[:])

    nc.compile()
    return nc


_nc_cache = None


def _get_nc():
    global _nc_cache
    if _nc_cache is None:
        _nc_cache = _build()
    return _nc_cache


def kernel(adjacency, features, kernel, bias):
    nc = _get_nc()
    feats = np.ascontiguousarray(features.reshape(B * V, F), dtype=np.float32)
    adj = np.ascontiguousarray(adjacency.reshape(B * V, E * NN), dtype=np.int32)
    w = np.ascontiguousarray(kernel, dtype=np.float32)
    b = np.ascontiguousarray(bias.reshape(1, U), dtype=np.float32)
    in_maps = []
    for i in range(NCORES):
        s = slice(i * T, (i + 1) * T)
        in_maps.append({
            "features": feats[s],
            "adjacency": adj[s],
            "weight": w,
            "bias": b,
        })
    res = run_bass_kernel_spmd(nc, in_maps, list(range(NCORES)))
    out = np.concatenate([res.results[i]["out"] for i in range(NCORES)], axis=0)
    return out.reshape(B, V, U).astype(features.dtype)


if __name__ == "__main__":
    import io, contextlib, re
    nc = _build()
    buf = io.StringIO()
    with contextlib.redirect_stdout(buf):
        nc.print_concise(deps=True)
    bad = 0
    for line in buf.getvalue().splitlines():
        n = len(re.findall(r"wait:S\[", line))
        if n > 1:
            bad += 1
            if bad <= 8:
                print("MULTI-WAIT:", line[:200])
    print(f"instructions with >1 wait: {bad}")



# revision 4
# speedup vs baseline: 1.6419x; 1.6419x over previous
"""Trainium2 Bass kernel for nn_ConvGraphSelfLoop (transposed formulation).

out = where(any(adj>=0, axes -1,-2), relu(features @ W + b), features)

Sharding: B*V = 65536 vertices split across 8 NeuronCores (8192 each);
W/bias replicated; no cross-core communication.

Layout trick: all device compute happens in *transposed* space.
The host ships features^T [F, T] (bf16) and adjacency^T [E*N, T]; the
device produces out^T [U, T] and the host transposes back. Benefits:
  - matmul lhsT (stationary) = native W[f,u] chunks -> zero PE transposes
  - psum partition dim is u -> bias is a per-partition ACT scalar (no
    bias matmuls); the main GEMM is the only PE work (+1 small mask mm)
  - invalid-vertex passthrough is a per-column predicate: one DVE
    copy_predicated per u-chunk, reading the same bf16 x tile the PE
    streams
Per 512-token sub-block: count valid adjacency entries with a ones[128]
stationary matmul over (adjT >= 0), invmask = Relu(1 - count) on ACT,
8 u-chunks x 8 f-chunk accumulated bf16 matmuls, ACT Relu+bias evict,
DVE predicated overwrite of invalid columns with x.
"""
import numpy as np
import ml_dtypes
import concourse.bass as bass
import concourse.bacc as bacc
import concourse.mybir as mybir
import concourse.tile as tile
from concourse.bass_utils import run_bass_kernel_spmd

B, V, E, NN = 4, 16384, 4, 32
F, U = 1024, 1024
A = E * NN                   # 128 adjacency entries per vertex
NCORES = 8
T = B * V // NCORES          # 8192 tokens per core
P = 128
G = 8                        # token groups per core (1024 tokens each)
GT = T // G                  # 1024 tokens per group
S = 2                        # 512-token sub-blocks per group
ST = GT // S                 # 512
C = F // P                   # 8 contraction (f) chunks
J = U // P                   # 8 output (u) chunks

f32 = mybir.dt.float32
bf16 = mybir.dt.bfloat16
i32 = mybir.dt.int32
AF = mybir.ActivationFunctionType
ALU = mybir.AluOpType


def _build():
    nc = bacc.Bacc("TRN2", target_bir_lowering=False, debug=False,
                   num_devices=NCORES)
    xT_d = nc.dram_tensor("xT", [F, T], bf16, kind="ExternalInput")
    adjT_d = nc.dram_tensor("adjT", [A, T], i32, kind="ExternalInput")
    w_d = nc.dram_tensor("w", [F, U], bf16, kind="ExternalInput")
    bias_d = nc.dram_tensor("bias", [P, J], f32, kind="ExternalInput")
    outT_d = nc.dram_tensor("outT", [U, T], f32, kind="ExternalOutput")

    with tile.TileContext(nc) as tc:
        with tc.tile_pool(name="const", bufs=1) as const, \
             tc.tile_pool(name="xg", bufs=2) as xp, \
             tc.tile_pool(name="ag", bufs=2) as apool, \
             tc.tile_pool(name="tv", bufs=2) as tvp, \
             tc.tile_pool(name="im", bufs=3) as imp, \
             tc.tile_pool(name="ot", bufs=3) as op, \
             tc.tile_pool(name="psM", bufs=2, space="PSUM") as psM, \
             tc.tile_pool(name="psO", bufs=3, space="PSUM") as psO:

            # ---- startup constants ----
            w_sb = const.tile([P, C, U], bf16)
            for c in range(C):
                nc.sync.dma_start(w_sb[:, c, :], w_d.ap()[c * P:(c + 1) * P, :])
            bias_sb = const.tile([P, J], f32)
            nc.sync.dma_start(bias_sb[:], bias_d.ap())
            ones_f = const.tile([P, P], f32)
            nc.gpsimd.memset(ones_f[:], 1.0)
            ones_b = const.tile([P, P], bf16)
            nc.scalar.copy(ones_b[:], ones_f[:])

            for g in range(G):
                gsl = slice(g * GT, (g + 1) * GT)
                # ---- DMA loads ----
                xg = xp.tile([P, C, GT], bf16, tag="xg")
                for c in range(C):
                    nc.sync.dma_start(xg[:, c, :],
                                      xT_d.ap()[c * P:(c + 1) * P, gsl])
                ag = apool.tile([P, GT], i32, tag="ag")
                nc.sync.dma_start(ag[:], adjT_d.ap()[:, gsl])

                # ---- DVE: valid-entry indicator (bf16 0/1) ----
                tv = tvp.tile([P, GT], bf16, tag="tv")
                nc.vector.tensor_scalar(tv[:], ag[:], 0, None, ALU.is_ge)

                for s in range(S):
                    ts = slice(s * ST, (s + 1) * ST)
                    # ---- PE: valid count per token (bcast to all parts) ----
                    pm = psM.tile([P, ST], f32, tag="pm")
                    nc.tensor.matmul(pm[:], ones_b[:], tv[:, ts],
                                     start=True, stop=True)
                    # ---- DVE: invalid mask (uint8) = (count <= 0) ----
                    im = imp.tile([P, ST], mybir.dt.uint8, tag="im")
                    nc.vector.tensor_scalar(im[:], pm[:], 0.0, None, ALU.is_le)

                    ot = op.tile([P, J, ST], f32, tag="ot")
                    for j in range(J):
                        pj = psO.tile([P, ST], f32, tag="pj")
                        for c in range(C):
                            nc.tensor.matmul(
                                pj[:],
                                w_sb[:, c, j * P:(j + 1) * P],
                                xg[:, c, ts],
                                start=(c == 0), stop=(c == C - 1))
                        # ---- ACT: relu(psum + bias_j) -> out tile ----
                        nc.scalar.activation(ot[:, j, :], pj[:], AF.Relu,
                                             bias=bias_sb[:, j:j + 1],
                                             scale=1.0)
                        # ---- DVE: overwrite invalid columns with x ----
                        nc.vector.copy_predicated(ot[:, j, :], im[:],
                                                  xg[:, j, ts])
                    # ---- DMA store ----
                    tabs = slice(g * GT + s * ST, g * GT + (s + 1) * ST)
                    for j in range(J):
                        nc.sync.dma_start(outT_d.ap()[j * P:(j + 1) * P, tabs],
                                          ot[:, j, :])

    nc.compile()
    return nc


_nc_cache = None


def _get_nc():
    global _nc_cache
    if _nc_cache is None:
        _nc_cache = _build()
    return _nc_cache


def prepare_in_maps(adjacency, features, kernel, bias):
    feats = features.reshape(B * V, F).astype(ml_dtypes.bfloat16)
    featsT = np.ascontiguousarray(feats.T)                    # [F, B*V] bf16
    adjT = np.ascontiguousarray(
        adjacency.reshape(B * V, A).astype(np.int32).T)       # [A, B*V]
    w = kernel.astype(ml_dtypes.bfloat16)                     # [F, U]
    b = np.ascontiguousarray(
        bias.reshape(J, P).astype(np.float32).T)              # [P, J]
    in_maps = []
    for i in range(NCORES):
        s = slice(i * T, (i + 1) * T)
        in_maps.append({
            "xT": np.ascontiguousarray(featsT[:, s]),
            "adjT": np.ascontiguousarray(adjT[:, s]),
            "w": w,
            "bias": b,
        })
    return in_maps


def kernel(adjacency, features, kernel, bias):
    nc = _get_nc()
    in_maps = prepare_in_maps(adjacency, features, kernel, bias)
    res = run_bass_kernel_spmd(nc, in_maps, list(range(NCORES)))
    outT = np.concatenate([res.results[i]["outT"] for i in range(NCORES)],
                          axis=1)                             # [U, B*V]
    out = np.ascontiguousarray(outT.T).reshape(B, V, U)
    return out.astype(features.dtype)


if __name__ == "__main__":
    import io, contextlib, re
    nc = _build()
    buf = io.StringIO()
    with contextlib.redirect_stdout(buf):
        nc.print_concise(deps=True)
    bad = 0
    for line in buf.getvalue().splitlines():
        n = len(re.findall(r"wait:S\[", line))
        if n > 1:
            bad += 1
            if bad <= 8:
                print("MULTI-WAIT:", line[:200])
    print(f"instructions with >1 wait: {bad}")
